# revision 1
# baseline (speedup 1.0000x reference)
"""Multi-head attention (B=2, S=2048, D=1024, H=16) on 8 TRN2 NeuronCores.

Sharding: tensor-parallel over heads x data-parallel over batch.
Core c handles batch b = c//4 and heads 4*(c%4) .. 4*(c%4)+3 (256 of the
1024 projected dims).  wq/wk/wv are split column-wise (rows of the [out,in]
weight), wo row-wise; each core emits a [S, D] partial of the output
projection and the host sums the 4 partials per batch.

Per-core kernel (all matmuls bf16, fp32 PSUM accumulation):
  1. Q^T, K^T [256, S] and V^T [256, S] projections (contraction over
     D=1024 in 8 chunks of 128; stationary = pre-transposed weight slices).
  2. V^T transposed back per 128-row chunk via PE transpose into a padded
     per-stack layout [v_h0 | 1 | 0..0 | v_h1] so each head's attnV matmul
     carries a ones-column that accumulates the softmax denominator.
  3. Per head: S^T[sk,sq] = K_h @ Q_h^T (K=64), P^T = exp(S^T/8) on ScalarE
     (PSUM->SBUF bf16), P^T *= mask^T (VectorE, bf16), O^T accumulated over
     sk chunks (K=128).  No max-subtraction: |scores/8| <~ 2.5 so exp is
     safely bounded, and masking multiplies by exact 0/1 after exp.
  4. ctx^T = O^T * (1/rowsum): bf16 reciprocal of the sums row, broadcast
     across partitions by a K=1 ones-row matmul, multiplied on VectorE.
  5. partial = ctx @ wo_slice^T via ctx^T-stationary matmuls, fp32 out; the
     stack-0 half is interleaved into heads 2/3, stack-1 DMA-accumulates.
"""

import sys

if "/opt/trn_rl_repo" not in sys.path:
    sys.path.insert(0, "/opt/trn_rl_repo")

import numpy as np
import ml_dtypes

B = 2
S = 2048
D = 1024
H = 16
DH = 64
P = 128
N_CORES = 8
HEADS_PER_CORE = 4
CORE_DIMS = HEADS_PER_CORE * DH  # 256
SQT = 512  # matmul moving free dim / PSUM bank
KC = D // P  # 8 contraction chunks for the input projections
BF16 = ml_dtypes.bfloat16

_CACHE = {}


def _build(s=S):
    """Build the single-core Bass program (same program on all 8 cores).

    Order: V projection (both stacks) -> V transposes -> Q -> K projections,
    then the four heads; the stack-0 output projection is interleaved into
    heads 2/3 as per-chunk fillers, the stack-1 half DMA-accumulates at the
    end.  Softmax normalization rides a ones-column through the attnV matmul
    (denominator), a bf16 reciprocal row, and a K=1 ones-matmul broadcast.
    """
    import concourse.bass as bass
    import concourse.bacc as bacc
    import concourse.mybir as mybir
    import concourse.tile as tile
    from concourse.masks import make_identity
    from contextlib import ExitStack

    dt = mybir.dt
    AF = mybir.ActivationFunctionType
    nsq = s // SQT  # Sq tiles
    nsk = s // P  # Sk chunks
    ntile = s // P
    sqg = [tuple(range(i, min(i + 2, nsq))) for i in range(0, nsq, 2)]
    VW = 208  # per-stack padded V row: [v_h0(64) | 1A | 1B | 0*62 | v_h1(64) | pad]
    # A-region lhsT (head hh=0): cols 0..64   = [v_h0 | ones]        M=65
    # B-region lhsT (head hh=1): cols 65..192 = [ones | 0*63 | v_h1] M=128

    nc = bacc.Bacc("TRN2", target_bir_lowering=False, debug=False)
    xqT = nc.declare_dram_parameter("xqT", [D, s], dt.bfloat16, isOutput=False)
    xkT = nc.declare_dram_parameter("xkT", [D, s], dt.bfloat16, isOutput=False)
    xvT = nc.declare_dram_parameter("xvT", [D, s], dt.bfloat16, isOutput=False)
    maskT = nc.declare_dram_parameter("maskT", [s, s], dt.bfloat16, isOutput=False)
    wqT = nc.declare_dram_parameter("wqT", [D, CORE_DIMS], dt.bfloat16, isOutput=False)
    wkT = nc.declare_dram_parameter("wkT", [D, CORE_DIMS], dt.bfloat16, isOutput=False)
    wvT = nc.declare_dram_parameter("wvT", [D, CORE_DIMS], dt.bfloat16, isOutput=False)
    woT = nc.declare_dram_parameter("woT", [CORE_DIMS, D], dt.bfloat16, isOutput=False)
    out = nc.declare_dram_parameter("out", [s, D], dt.float32, isOutput=True)

    with ExitStack() as ctx:
        tc = ctx.enter_context(tile.TileContext(nc))
        consts = ctx.enter_context(tc.tile_pool(name="consts", bufs=1))
        xpool = ctx.enter_context(tc.tile_pool(name="xpool", bufs=11))
        wpool = ctx.enter_context(tc.tile_pool(name="wpool", bufs=1))
        mpool = ctx.enter_context(tc.tile_pool(name="mpool", bufs=1))
        actpool = ctx.enter_context(tc.tile_pool(name="actpool", bufs=1))
        ptpool = ctx.enter_context(tc.tile_pool(name="ptpool", bufs=5))
        rpool = ctx.enter_context(tc.tile_pool(name="rpool", bufs=2))
        oqpool = ctx.enter_context(tc.tile_pool(name="oqpool", bufs=2))
        opool = ctx.enter_context(tc.tile_pool(name="opool", bufs=3))
        psA = ctx.enter_context(tc.tile_pool(name="psA", bufs=2, space="PSUM"))
        psB = ctx.enter_context(tc.tile_pool(name="psB", bufs=4, space="PSUM"))

        ident = consts.tile([P, P], dt.bfloat16)
        make_identity(nc, ident)
        onesb = consts.tile([P, P], dt.bfloat16)
        nc.vector.memset(onesb, 1.0)
        # warm the Exp activation table off the critical path
        warm = consts.tile([P, 1], dt.float32)
        nc.vector.memset(warm, 0.0)
        nc.scalar.activation(warm, warm, AF.Exp, scale=1.0)

        # --- resident weights ---
        wq_sb = wpool.tile([P, KC, CORE_DIMS], dt.bfloat16, tag="wq")
        wk_sb = wpool.tile([P, KC, CORE_DIMS], dt.bfloat16, tag="wk")
        wv_sb = wpool.tile([P, KC, CORE_DIMS], dt.bfloat16, tag="wv")
        wo_sb = wpool.tile([P, 2, D], dt.bfloat16, tag="wo")
        for wsb, wdr in ((wv_sb, wvT), (wq_sb, wqT), (wk_sb, wkT)):
            nc.gpsimd.dma_start(
                out=wsb, in_=wdr.rearrange("(kc p) m -> p kc m", p=P)
            )
        nc.gpsimd.dma_start(out=wo_sb, in_=woT.rearrange("(st p) n -> p st n", p=P))

        # --- projection outputs (transposed: [stack-dim 128, stack, s]) ---
        qT_sb = actpool.tile([P, 2, s], dt.bfloat16, tag="qT")
        kT_sb = actpool.tile([P, 2, s], dt.bfloat16, tag="kT")
        vT_sb = actpool.tile([P, 2, s], dt.bfloat16, tag="vT")
        ctxT_sb = actpool.tile([P, 2, s], dt.bfloat16, tag="ctxT")
        vpad = actpool.tile([P, nsk, 2, VW], dt.bfloat16, tag="vpad")

        # init vpad: zeros everywhere, ones at cols 64 (A) and 65 (B)
        nc.vector.memset(vpad, 0.0)
        nc.vector.memset(vpad[:, :, :, 64:66], 1.0)

        mask_sb = mpool.tile([P, nsk, s], dt.bfloat16, tag="mask")

        def load_x(xdr):
            xch = []
            for kc in range(KC):
                xt = xpool.tile([P, s], dt.bfloat16, name="x", tag="x")
                half = s // 2
                nc.gpsimd.dma_start(
                    out=xt[:, 0:half], in_=xdr[kc * P:(kc + 1) * P, 0:half]
                )
                nc.gpsimd.dma_start(
                    out=xt[:, half:s], in_=xdr[kc * P:(kc + 1) * P, half:s]
                )
                xch.append(xt)
            return xch

        def proj(xch, wsb, osb, st):
            pp = [None] * len(sqg)
            for kc in range(KC):
                lhsT = wsb[:, kc, st * P:(st + 1) * P]
                for gi, grp in enumerate(sqg):
                    if kc == 0:
                        pp[gi] = psA.tile(
                            [P, len(grp) * SQT], dt.float32, name="psA", tag="psA"
                        )
                    for j, sq in enumerate(grp):
                        nc.tensor.matmul(
                            pp[gi][:, j * SQT:(j + 1) * SQT],
                            lhsT,
                            xch[kc][:, sq * SQT:(sq + 1) * SQT],
                            start=(kc == 0),
                            stop=(kc == KC - 1),
                        )
            for gi, grp in enumerate(sqg):
                nc.vector.tensor_copy(
                    osb[:, st, grp[0] * SQT:(grp[-1] + 1) * SQT], pp[gi]
                )

        def all_proj():
            xch = load_x(xvT)
            for st in range(2):
                proj(xch, wv_sb, vT_sb, st)
            for st in range(2):
                for c in range(nsk):
                    pst = psB.tile([P, P], dt.bfloat16, name="pst", tag="psB")
                    nc.tensor.transpose(
                        pst, vT_sb[:, st, c * P:(c + 1) * P], ident
                    )
                    nc.scalar.copy(vpad[:, c, st, 0:64], pst[:, 0:64])
                    nc.scalar.copy(vpad[:, c, st, 129:193], pst[:, 64:128])
            xch = load_x(xqT)
            for st in range(2):
                proj(xch, wq_sb, qT_sb, st)
            xch = load_x(xkT)
            for c in range(nsk):
                nc.gpsimd.dma_start(
                    out=mask_sb[:, c, :], in_=maskT[c * P:(c + 1) * P, :]
                )
            for st in range(2):
                proj(xch, wk_sb, kT_sb, st)

        def outproj_stile(st, stile):
            accum = (
                mybir.AluOpType.bypass if st == 0 else mybir.AluOpType.add
            )
            pp = psA.tile([P, 2 * SQT], dt.float32, name="psA", tag="psA")
            lhsT = ctxT_sb[:, st, stile * P:(stile + 1) * P]
            for oh in range(2):
                nc.tensor.matmul(
                    pp[:, oh * SQT:(oh + 1) * SQT],
                    lhsT,
                    wo_sb[:, st, oh * SQT:(oh + 1) * SQT],
                    start=True,
                    stop=True,
                )
            ob = opool.tile([P, 2 * SQT], dt.float32, name="ob", tag="ob")
            if st == 0:
                nc.vector.tensor_copy(ob, pp)
            else:
                nc.scalar.copy(ob, pp)
            for oh in range(2):
                nc.gpsimd.dma_start(
                    out=out[stile * P:(stile + 1) * P, oh * SQT:(oh + 1) * SQT],
                    in_=ob[:, oh * SQT:(oh + 1) * SQT],
                    accum_op=accum,
                )

        def attention(h, fillers=()):
            st, hh = h // 2, h % 2
            hp = hh * 64  # partition base of this head inside the stack
            po = [
                psB.tile([P, SQT], dt.float32, name="psBo", tag="psB")
                for _ in range(nsq)
            ]
            fillers = list(fillers)
            for c in range(nsk):
                if fillers:
                    fillers.pop(0)()
                lhs_k = kT_sb[hp:hp + 64, st, c * P:(c + 1) * P]
                ps = [None] * len(sqg)
                for gi, grp in enumerate(sqg):
                    ps[gi] = psA.tile(
                        [P, len(grp) * SQT], dt.float32, name="psA", tag="psA"
                    )
                    for j, sq in enumerate(grp):
                        nc.tensor.matmul(
                            ps[gi][:, j * SQT:(j + 1) * SQT],
                            lhs_k,
                            qT_sb[hp:hp + 64, st, sq * SQT:(sq + 1) * SQT],
                            start=True,
                            stop=True,
                        )
                pt = ptpool.tile([P, s], dt.bfloat16, name="pt", tag="pt")
                for gi, grp in enumerate(sqg):
                    nc.scalar.activation(
                        pt[:, grp[0] * SQT:(grp[-1] + 1) * SQT], ps[gi], AF.Exp,
                        scale=0.125,
                    )
                nc.vector.tensor_mul(pt, pt, mask_sb[:, c, :])
                if hh == 0:
                    lhs_v = vpad[:, c, st, 0:65]  # [v|1] -> out parts 0..64
                    mrows = 65
                else:
                    lhs_v = vpad[:, c, st, 65:193]  # [1|0..|v] -> out parts 0..127
                    mrows = P
                for sq in range(nsq):
                    nc.tensor.matmul(
                        po[sq][0:mrows, :],
                        lhs_v,
                        pt[:, sq * SQT:(sq + 1) * SQT],
                        start=(c == 0),
                        stop=(c == nsk - 1),
                    )
            while fillers:
                fillers.pop(0)()
            # quick-release PSUM, then normalize: ctxT = O^T * (1/rowsum)
            srow = 64 if hh == 0 else 0  # partition holding the sums
            orow = 0 if hh == 0 else 64  # partition base of O^T rows
            mrows = 65 if hh == 0 else P
            for sq in range(nsq):
                oq = oqpool.tile([P, SQT], dt.float32, name="oq", tag="oq")
                nc.vector.tensor_copy(oq[0:mrows, :], po[sq][0:mrows, :])
                r = rpool.tile([P, SQT], dt.bfloat16, name="r", tag="r")
                with nc.allow_low_precision(reason="softmax denom bcast in bf16"):
                    nc.vector.reciprocal(r[srow:srow + 1, :], oq[srow:srow + 1, :])
                # broadcast 1/rowsum to all partitions via a K=1 ones matmul
                rb = psB.tile([P, SQT], dt.float32, name="psBr", tag="psB")
                nc.tensor.matmul(
                    rb,
                    onesb[srow:srow + 1, :],
                    r[srow:srow + 1, :],
                    start=True,
                    stop=True,
                )
                nc.vector.tensor_mul(
                    ctxT_sb[hp:hp + 64, st, sq * SQT:(sq + 1) * SQT],
                    oq[orow:orow + 64, :],
                    rb[orow:orow + 64, :],
                )

        all_proj()
        attention(0)
        attention(1)
        mk = lambda st, i: (lambda: outproj_stile(st, i))
        attention(2, fillers=[mk(0, i) for i in range(0, ntile // 2)])
        attention(3, fillers=[mk(0, i) for i in range(ntile // 2, ntile)])
        for i in range(ntile):
            outproj_stile(1, i)

    nc.compile()
    return nc



def _shard_inputs(query, key, value, mask, wq, wk, wv, wo):
    query = np.asarray(query, dtype=np.float32)
    key = np.asarray(key, dtype=np.float32)
    value = np.asarray(value, dtype=np.float32)
    mask = np.asarray(mask)
    wq = np.asarray(wq, dtype=np.float32)
    wk = np.asarray(wk, dtype=np.float32)
    wv = np.asarray(wv, dtype=np.float32)
    wo = np.asarray(wo, dtype=np.float32)

    xT = []
    mT = []
    for b in range(B):
        xT.append(
            tuple(
                np.ascontiguousarray(a[b].T).astype(BF16)
                for a in (query, key, value)
            )
        )
        mT.append(np.ascontiguousarray(mask[b].T).astype(BF16))

    in_maps = []
    for c in range(N_CORES):
        b, g = c // 4, c % 4
        hsel = slice(g * CORE_DIMS, (g + 1) * CORE_DIMS)
        in_maps.append(
            {
                "xqT": xT[b][0],
                "xkT": xT[b][1],
                "xvT": xT[b][2],
                "maskT": mT[b],
                "wqT": np.ascontiguousarray(wq[hsel].T).astype(BF16),
                "wkT": np.ascontiguousarray(wk[hsel].T).astype(BF16),
                "wvT": np.ascontiguousarray(wv[hsel].T).astype(BF16),
                "woT": np.ascontiguousarray(wo[:, hsel].T).astype(BF16),
            }
        )
    return in_maps


LAST_RESULTS = None  # BassKernelResults of the most recent kernel() call


def kernel(query, key, value, mask, wq, wk, wv, wo):
    global LAST_RESULTS
    from concourse import bass_utils

    if "nc" not in _CACHE:
        _CACHE["nc"] = _build()
    nc = _CACHE["nc"]

    in_maps = _shard_inputs(query, key, value, mask, wq, wk, wv, wo)
    res = bass_utils.run_bass_kernel_spmd(nc, in_maps, core_ids=list(range(N_CORES)))
    LAST_RESULTS = res

    outp = np.empty((B, S, D), dtype=np.float32)
    for b in range(B):
        acc = res.results[4 * b]["out"].astype(np.float32)
        for g in range(1, 4):
            acc = acc + res.results[4 * b + g]["out"]
        outp[b] = acc
    return outp

